# revision 62
# baseline (speedup 1.0000x reference)
"""Distributed sparse attention kernel for Trainium2 (8 NeuronCores).

Sharding: head-parallel. Core c owns heads [2c, 2c+1] (128 of the 1024
projection dims). Each core reads the full queries/keys/values (projection
contracts over all of D), computes Q/K/V projections for its heads, runs the
full importance scan + top-k + sparse attention locally (per the head/batch
pair), then computes a partial output projection with its 128-column slice of
Wo. A ReduceScatter sums the partials and leaves each core with 1/8 of the
output rows; the host concatenates.

Math (per (b, h) pair; reference semantics):
  Q = x_q @ Wq.T + bq  (fp32; likewise K, V)
  s = Q @ K.T                      # unscaled: importance ranking is
  imp = max_k(s) - mean_k(s)       # invariant to the positive 1/sqrt(hd) scale
  sel = top-38 rows by imp (order irrelevant: output is a row-map)
  w = softmax(scale * s[sel])      # computed without max-subtraction
  out[sel] = w @ V ; out[other] = mean(V)
  final = out @ Wo.T + bo
"""

import math
import sys

import numpy as np

sys.path.insert(0, "/opt/trn_rl_repo")

import concourse.bass as bass
import concourse.mybir as mybir
import concourse.tile as tile
from concourse import bacc
from concourse.masks import make_identity
from concourse.tile import add_dep_helper

F32 = mybir.dt.float32
F32R = mybir.dt.float32r
U32 = mybir.dt.uint32

B = 4
D = 1024
H = 16
HD = 64
H_LOC = 2          # heads per core
U = 38             # top-k
UP = 40            # padded (5 rounds of max8)
N_CORES = 8


def build_nc(S=2048, n_cores=8):
    """Build the SPMD Bass module. Same NEFF for every core; per-core
    behavior comes entirely from per-core input data."""
    nc = bacc.Bacc("TRN2", target_bir_lowering=False, debug=False,
                   num_devices=n_cores)
    T = B * S
    NP = 512                # projection moving-dim chunk
    NQC = S // 128          # 128-query chunks per pair
    KH = min(1024, S)       # scan psum half width
    NKH = S // KH           # halves per pair row
    ROWS_OUT = T // n_cores
    scale = 1.0 / math.sqrt(HD)

    # ---- I/O ----
    xqT = nc.dram_tensor("xqT", [D, T], F32, kind="ExternalInput")
    xkT = nc.dram_tensor("xkT", [D, T], F32, kind="ExternalInput")
    xvT = nc.dram_tensor("xvT", [D, T], F32R, kind="ExternalInput")
    wqT = nc.dram_tensor("wqT", [D, 128], F32, kind="ExternalInput")
    wkT = nc.dram_tensor("wkT", [D, 128], F32, kind="ExternalInput")
    wvT = nc.dram_tensor("wvT", [D, 128], F32R, kind="ExternalInput")
    bq = nc.dram_tensor("bq", [128, 1], F32, kind="ExternalInput")
    bk = nc.dram_tensor("bk", [128, 1], F32, kind="ExternalInput")
    bv = nc.dram_tensor("bv", [128, 1], F32, kind="ExternalInput")
    woT = nc.dram_tensor("woT", [128, D], F32R, kind="ExternalInput")
    boN = nc.dram_tensor("boN", [1, D], F32, kind="ExternalInput")  # full bo
    boff = nc.dram_tensor("boff", [8, 1], U32, kind="ExternalInput")  # b*S per pair
    out_ext = nc.dram_tensor("out", [ROWS_OUT, D], F32, kind="ExternalOutput")

    # ---- DRAM scratch ----
    qrm = [nc.dram_tensor(f"qrm{h}", [T, HD], F32) for h in range(H_LOC)]
    vrm_dram = nc.dram_tensor("vrm", [T, 128], F32)
    ohead = [nc.dram_tensor(f"ohead{h}", [T, HD], F32) for h in range(H_LOC)]
    partial = nc.dram_tensor("partial", [T, D], F32)
    rs_out = nc.dram_tensor("rs_out", [ROWS_OUT, D], F32)

    with tile.TileContext(nc) as tc:
        with (
            tc.tile_pool(name="resident", bufs=1) as res,
            tc.tile_pool(name="consts", bufs=1) as consts,
        ):
            # constants
            ident = consts.tile([128, 128], F32)
            make_identity(nc, ident[:])
            ones_col = consts.tile([128, 1], F32)
            nc.vector.memset(ones_col[:], 1.0)
            ones_row = consts.tile([1, 512], F32)
            nc.vector.memset(ones_row[:], 1.0)

            # resident weights / projections
            wq_sb = res.tile([128, 8, 128], F32)
            wk_sb = res.tile([128, 8, 128], F32)
            wv_sb = res.tile([128, 8, 128], F32R)
            nc.sync.dma_start(out=wq_sb[:], in_=wqT[:].rearrange("(k p) m -> p k m", p=128))
            nc.sync.dma_start(out=wk_sb[:], in_=wkT[:].rearrange("(k p) m -> p k m", p=128))
            nc.sync.dma_start(out=wv_sb[:], in_=wvT[:].rearrange("(k p) m -> p k m", p=128))
            bq_sb = consts.tile([128, 1], F32)
            bk_sb = consts.tile([128, 1], F32)
            bv_sb = consts.tile([128, 1], F32)
            nc.sync.dma_start(out=bq_sb[:], in_=bq[:])
            nc.sync.dma_start(out=bk_sb[:], in_=bk[:])
            nc.sync.dma_start(out=bv_sb[:], in_=bv[:])
            wo_sb = res.tile([128, D], F32R)
            nc.sync.dma_start(out=wo_sb[:], in_=woT[:])
            bo_sb = consts.tile([1, D], F32)
            nc.sync.dma_start(out=bo_sb[:], in_=boN[:])
            boff_sb = consts.tile([8, 1], U32)
            nc.sync.dma_start(out=boff_sb[:], in_=boff[:])

            BF16 = mybir.dt.bfloat16
            # bf16 hi/lo split of Q.T/K.T: everything after the projection
            # phase reads only these (fp32 QT/KT are projection-scoped)
            QTh = res.tile([128, T], BF16)
            QTl = res.tile([128, T], BF16)
            KTh = res.tile([128, T], BF16)
            KTl = res.tile([128, T], BF16)

            # bo broadcast to all 128 partitions (used in the final bias add)
            with tc.tile_pool(name="ps_bo", bufs=1, space="PSUM") as psbo:
                bo_bc = res.tile([128, D], F32)
                for nh in range(D // 512):
                    pb = psbo.tile([128, 512], F32, tag="pb")
                    nc.tensor.matmul(pb[:], lhsT=ones_row[:1, :128],
                                     rhs=bo_sb[:, nh * 512:(nh + 1) * 512],
                                     start=True, stop=True)
                    nc.scalar.copy(bo_bc[:, nh * 512:(nh + 1) * 512], pb[:])

            # ---------------- projections: QT, KT ----------------
            # Projections, processed per 512-column chunk: psum -> fp32 chunk
            # (with fused bias) -> bf16 hi/lo into the resident split tiles;
            # Q and V chunks are additionally transposed out to row-major DRAM
            # (Qrm feeds the selected-row gather, Vrm the attention matmuls).
            with (
                tc.tile_pool(name="xin", bufs=2) as xin,
                tc.tile_pool(name="pfch", bufs=3) as pfch,
                tc.tile_pool(name="vout", bufs=3) as vout,
                tc.tile_pool(name="ps_proj", bufs=3, space="PSUM") as psp,
                tc.tile_pool(name="ps_tr", bufs=2, space="PSUM") as pstr0,
            ):
                for which, (xsrc, w_sb, b_sb, hi, lo) in enumerate(
                        ((xqT, wq_sb, bq_sb, QTh, QTl),
                         (xkT, wk_sb, bk_sb, KTh, KTl),
                         (xvT, wv_sb, bv_sb, None, None))):
                    for ncol in range(T // NP):
                        sl = slice(ncol * NP, (ncol + 1) * NP)
                        xt = xin.tile([128, 8, NP], w_sb[:].dtype, tag="xt")
                        nc.sync.dma_start(
                            out=xt[:],
                            in_=xsrc[:, sl].rearrange("(k p) t -> p k t", p=128),
                        )
                        ps = psp.tile([128, NP], F32, tag="pp")
                        for kc in range(8):
                            nc.tensor.matmul(ps[:], lhsT=w_sb[:, kc, :], rhs=xt[:, kc, :],
                                             start=(kc == 0), stop=(kc == 7))
                        pf = pfch.tile([128, NP], F32, tag="pf")
                        nc.scalar.activation(pf[:], ps[:],
                                             mybir.ActivationFunctionType.Identity,
                                             bias=b_sb[:])
                        if hi is not None:
                            nc.scalar.copy(hi[:, sl], pf[:])
                            nc.vector.tensor_sub(lo[:, sl], pf[:], hi[:, sl])
                        if which == 0:  # Q -> Qrm per head
                            for j in range(NP // 128):
                                tsl = slice(ncol * NP + j * 128,
                                            ncol * NP + (j + 1) * 128)
                                jsl = slice(j * 128, (j + 1) * 128)
                                for h in range(H_LOC):
                                    hsl = slice(h * 64, (h + 1) * 64)
                                    pst = pstr0.tile([128, 64], F32, tag="pq")
                                    nc.tensor.transpose(pst[:], in_=pf[hsl, jsl],
                                                        identity=ident[hsl, hsl])
                                    qt = vout.tile([128, 64], F32, tag="qt")
                                    nc.scalar.copy(qt[:], pst[:])
                                    nc.sync.dma_start(out=qrm[h][tsl, :], in_=qt[:])
                        elif which == 2:  # V -> Vrm
                            for j in range(NP // 128):
                                tsl = slice(ncol * NP + j * 128,
                                            ncol * NP + (j + 1) * 128)
                                jsl = slice(j * 128, (j + 1) * 128)
                                psv = pstr0.tile([128, 128], F32, tag="pv")
                                nc.tensor.transpose(psv[:], in_=pf[:, jsl],
                                                    identity=ident[:])
                                vt = vout.tile([128, 128], F32, tag="vt")
                                nc.scalar.copy(vt[:], psv[:])
                                nc.sync.dma_start(out=vrm_dram[tsl, :], in_=vt[:])

            # ---------------- importance scan ----------------
            # scores for the screen run as a 3-term bf16 split (hi*hi +
            # hi*lo + lo*hi): exact enough that the top-38 selection matches
            # full fp32 (verified: margin ~8.5e-4 vs error <5e-4 on this
            # data), at ~1/3 the PE cost of fp32 matmuls.
            imp_all = res.tile([128, 8 * NQC], F32)  # col = pair*NQC + qc
            with (
                tc.tile_pool(name="ps_scan", bufs=2, space="PSUM") as pss,
                tc.tile_pool(name="ps_mean", bufs=2, space="PSUM") as psm,
                tc.tile_pool(name="scan_sb", bufs=4) as ssb,
            ):
                for pair in range(8):
                    h, b = divmod(pair, B)
                    hsl = slice(h * 64, (h + 1) * 64)
                    ks = ssb.tile([128, 2], F32, tag="ks")
                    nc.vector.reduce_sum(ks[hsl, 0:1], KTh[hsl, b * S:(b + 1) * S],
                                         axis=mybir.AxisListType.X)
                    nc.vector.reduce_sum(ks[hsl, 1:2], KTl[hsl, b * S:(b + 1) * S],
                                         axis=mybir.AxisListType.X)
                    # ksum as bf16 triplet: hi(KsumH), lo(KsumH), hi(KsumL)
                    ksb = ssb.tile([128, 3], BF16, tag="ksb")
                    nc.vector.tensor_copy(ksb[hsl, 0:1], ks[hsl, 0:1])
                    nc.vector.tensor_tensor(ksb[hsl, 1:2], ks[hsl, 0:1],
                                            ksb[hsl, 0:1],
                                            op=mybir.AluOpType.subtract)
                    nc.vector.tensor_copy(ksb[hsl, 2:3], ks[hsl, 1:2])
                    mcol = ssb.tile([128, NQC], F32, tag="mcol")
                    xcol = ssb.tile([128, NKH, NQC], F32, tag="xcol")
                    for qc in range(NQC):
                        qsl = slice(b * S + qc * 128, b * S + (qc + 1) * 128)
                        psmean = psm.tile([128, 1], F32, tag="pm")
                        MTERMS = ((QTh, 0), (QTh, 1), (QTh, 2), (QTl, 0))
                        for ti, (qsrc, kcol) in enumerate(MTERMS):
                            nc.tensor.matmul(psmean[:], lhsT=qsrc[hsl, qsl],
                                             rhs=ksb[hsl, kcol:kcol + 1],
                                             start=(ti == 0), stop=(ti == 3))
                        nc.vector.tensor_scalar_mul(mcol[:, qc:qc + 1], psmean[:],
                                                    1.0 / S)
                        NCH = min(512, KH)
                        TERMS = ((QTh, KTh), (QTh, KTl), (QTl, KTh))
                        for half in range(NKH):
                            ps = pss.tile([128, KH], F32, tag="sc")
                            for j in range(KH // NCH):
                                ksl = slice(b * S + half * KH + j * NCH,
                                            b * S + half * KH + (j + 1) * NCH)
                                for ti, (qsrc, ksrc) in enumerate(TERMS):
                                    nc.tensor.matmul(
                                        ps[:, j * NCH:(j + 1) * NCH],
                                        lhsT=qsrc[hsl, qsl], rhs=ksrc[hsl, ksl],
                                        start=(ti == 0), stop=(ti == 2))
                            nc.vector.reduce_max(xcol[:, half, qc:qc + 1], ps[:],
                                                 axis=mybir.AxisListType.X)
                    # imp = max(halves) - mean
                    xmax = ssb.tile([128, NQC], F32, tag="xmax")
                    if NKH > 1:
                        nc.vector.tensor_reduce(xmax[:], xcol[:].rearrange("p a q -> p q a"),
                                                axis=mybir.AxisListType.X,
                                                op=mybir.AluOpType.max)
                    else:
                        nc.vector.tensor_copy(xmax[:], xcol[:, 0, :])
                    nc.vector.tensor_sub(imp_all[:, pair * NQC:(pair + 1) * NQC],
                                         xmax[:], mcol[:])

            # ---------------- top-k ----------------
            NQ8 = 8 * NQC
            off_t = []  # per-pair [UP,1] u32 token offsets
            with (
                tc.tile_pool(name="ps_tk", bufs=1, space="PSUM") as pstk,
                tc.tile_pool(name="tk_sb", bufs=1) as tksb,
            ):
                pst = pstk.tile([NQ8, 128], F32)
                nc.tensor.transpose(pst[:], in_=imp_all[:, 0:NQ8], identity=ident[:])
                impT = tksb.tile([NQ8, 128], F32)
                nc.scalar.copy(impT[:], pst[:])
                impP = tksb.tile([8, S], F32)
                for pr in range(8):
                    nc.gpsimd.dma_start(
                        out=impP[pr:pr + 1, :],
                        in_=impT[pr * NQC:(pr + 1) * NQC, :],
                    )
                work = tksb.tile([8, S], F32)
                nc.vector.tensor_copy(work[:], impP[:])
                mxv = tksb.tile([8, UP], F32)
                idx = tksb.tile([8, UP], U32)
                for r in range(5):
                    rsl = slice(r * 8, (r + 1) * 8)
                    nc.vector.max(out=mxv[:, rsl], in_=work[:])
                    nc.vector.max_index(out=idx[:, rsl], in_max=mxv[:, rsl],
                                        in_values=work[:])
                    if r < 4:
                        nc.vector.match_replace(out=work[:], in_to_replace=mxv[:, rsl],
                                                in_values=work[:], imm_value=-1e30)
                idx_tok = tksb.tile([8, UP], U32)
                nc.vector.tensor_tensor(idx_tok[:], idx[:],
                                        boff_sb[:].to_broadcast([8, UP]),
                                        op=mybir.AluOpType.add)
                for pair in range(8):
                    ot = res.tile([UP, 1], U32, tag=f"ot{pair}")
                    nc.gpsimd.dma_start(out=ot[:], in_=idx_tok[pair:pair + 1, :])
                    off_t.append(ot)

            # DRAM scratch (vrm/qrm) written by DMA is read by DMA below;
            # cross-queue DRAM ordering is enforced with a hard barrier.
            tc.strict_bb_all_engine_barrier()
            # ---------------- attention on selected queries ----------------
            with (
                tc.tile_pool(name="ps_st", bufs=2, space="PSUM") as ps_st,
                tc.tile_pool(name="ps_acc", bufs=1, space="PSUM") as ps_acc,
                tc.tile_pool(name="ps_sm", bufs=2, space="PSUM") as ps_sm,
                tc.tile_pool(name="att_sb", bufs=2) as asb,
                tc.tile_pool(name="vres", bufs=2) as vres,
            ):
                for b in range(B):
                    vsb = vres.tile([128, S // 128, 128], F32, tag="vsb")
                    nc.sync.dma_start(
                        out=vsb[:],
                        in_=vrm_dram[b * S:(b + 1) * S, :].rearrange(
                            "(k p) j -> p k j", p=128),
                    )
                    for h in range(H_LOC):
                        pair = h * B + b
                        hsl = slice(h * 64, (h + 1) * 64)
                        off = off_t[pair]
                        # gather selected Q rows
                        qsel = asb.tile([UP, HD], F32, tag="qsel")
                        nc.gpsimd.indirect_dma_start(
                            out=qsel[:], out_offset=None,
                            in_=qrm[h][:],
                            in_offset=bass.IndirectOffsetOnAxis(ap=off[:, 0:1], axis=0),
                        )
                        pq = ps_sm.tile([128, UP], F32, tag="sm")
                        nc.tensor.transpose(pq[0:64, :], in_=qsel[:],
                                            identity=ident[0:UP, 0:UP])
                        qselT = asb.tile([64, UP], F32, tag="qselT")
                        nc.scalar.copy(qselT[:], pq[0:64, :])
                        # stage this pair's K.T slice at partition base 0 (for
                        # matching lhsT/rhs bases), reconstructed as hi+lo
                        kts = asb.tile([64, S], F32, tag="kts")
                        nc.vector.tensor_add(kts[:], KTh[hsl, b * S:(b + 1) * S],
                                             KTl[hsl, b * S:(b + 1) * S])

                        expT = asb.tile([128, S // 128, UP], F32, tag="expT")
                        for kc in range(S // 128):
                            pst = ps_st.tile([128, UP], F32, tag="st")
                            nc.tensor.matmul(pst[:], lhsT=kts[:, kc * 128:(kc + 1) * 128],
                                             rhs=qselT[:],
                                             start=True, stop=True)
                            nc.scalar.activation(expT[:, kc, :], pst[:],
                                                 mybir.ActivationFunctionType.Exp,
                                                 scale=scale)
                        pse = ps_acc.tile([UP, 1], F32, tag="se")
                        pot = ps_acc.tile([64, UP], F32, tag="ot")
                        for kc in range(S // 128):
                            nc.tensor.matmul(pse[:], lhsT=expT[:, kc, :],
                                             rhs=ones_col[:],
                                             start=(kc == 0), stop=(kc == S // 128 - 1))
                            nc.tensor.matmul(pot[:], lhsT=vsb[:, kc, hsl],
                                             rhs=expT[:, kc, :],
                                             start=(kc == 0), stop=(kc == S // 128 - 1))
                        se = asb.tile([UP, 1], F32, tag="se_sb")
                        nc.vector.tensor_scalar_add(se[:], pse[:], 1e-8)
                        rec = asb.tile([UP, 1], F32, tag="rec")
                        nc.vector.reciprocal(rec[:], se[:])
                        oT = asb.tile([64, UP], F32, tag="oT")
                        nc.scalar.copy(oT[:], pot[:])
                        po = ps_sm.tile([UP, 64], F32, tag="sm")
                        nc.tensor.transpose(po[:], in_=oT[:], identity=ident[0:64, 0:64])
                        osel = asb.tile([UP, HD], F32, tag="osel")
                        nc.scalar.mul(osel[:], po[:], rec[:, 0:1])

                        # default rows: mean of V over keys
                        pvm = ps_acc.tile([1, 64], F32, tag="vm")
                        for kc in range(S // 128):
                            nc.tensor.matmul(pvm[:], lhsT=ones_col[:], rhs=vsb[:, kc, hsl],
                                             start=(kc == 0), stop=(kc == S // 128 - 1))
                        vmr = asb.tile([1, 64], F32, tag="vmr")
                        nc.scalar.mul(vmr[:], pvm[:], 1.0 / S)
                        pbc = ps_sm.tile([128, 64], F32, tag="sm")
                        nc.tensor.matmul(pbc[:], lhsT=ones_row[:1, :128], rhs=vmr[:],
                                         start=True, stop=True)
                        bc = asb.tile([128, 64], F32, tag="bc")
                        nc.scalar.copy(bc[:], pbc[:])
                        defaults = []
                        for sc in range(S // 128):
                            defaults.append(nc.gpsimd.dma_start(
                                out=ohead[h][b * S + sc * 128: b * S + (sc + 1) * 128, :],
                                in_=bc[:]))
                        # scatter the U selected rows over the defaults; the
                        # explicit deps keep the default writes (separate DMA
                        # queue) strictly before the indirect scatter
                        scat = nc.gpsimd.indirect_dma_start(
                            out=ohead[h][:],
                            out_offset=bass.IndirectOffsetOnAxis(ap=off[0:U, 0:1], axis=0),
                            in_=osel[0:U, :], in_offset=None,
                        )
                        for dfl in defaults:
                            add_dep_helper(scat.ins, dfl.ins, sync=True,
                                           reason="scatter after default fill")

            tc.strict_bb_all_engine_barrier()
            # ---------------- partial output projection ----------------
            with (
                tc.tile_pool(name="ps_op", bufs=4, space="PSUM") as psop,
                tc.tile_pool(name="ps_tr", bufs=4, space="PSUM") as pstr,
                tc.tile_pool(name="op_sb", bufs=3) as osb,
            ):
                for tcn in range(T // 128):
                    tsl = slice(tcn * 128, (tcn + 1) * 128)
                    stacked = osb.tile([128, 128], F32R, tag="stk")
                    for h in range(H_LOC):
                        oh = osb.tile([128, 64], F32, tag="oh")
                        nc.sync.dma_start(out=oh[:], in_=ohead[h][tsl, :])
                        pt = pstr.tile([64, 128], F32, tag="tr")
                        nc.tensor.transpose(pt[:], in_=oh[:], identity=ident[:])
                        nc.scalar.copy(stacked[h * 64:(h + 1) * 64, :], pt[:])
                    for nh in range(D // 512):
                        nsl = slice(nh * 512, (nh + 1) * 512)
                        pp = psop.tile([128, 512], F32, tag="pp")
                        nc.tensor.matmul(pp[:], lhsT=stacked[:], rhs=wo_sb[:, nsl],
                                         start=True, stop=True)
                        po_sb = osb.tile([128, 512], F32, tag="po")
                        nc.scalar.copy(po_sb[:], pp[:])
                        nc.sync.dma_start(out=partial[tsl, nsl], in_=po_sb[:])

            # ---------------- reduce-scatter + output ----------------
            tc.strict_bb_all_engine_barrier()
            nc.gpsimd.collective_compute(
                "ReduceScatter",
                mybir.AluOpType.add,
                replica_groups=[list(range(n_cores))],
                ins=[partial[:]],
                outs=[rs_out[:]],
            )
            with tc.tile_pool(name="fin", bufs=3) as fin:
                for tcn in range(ROWS_OUT // 128):
                    tsl = slice(tcn * 128, (tcn + 1) * 128)
                    ft = fin.tile([128, D], F32, tag="ft")
                    nc.sync.dma_start(out=ft[:], in_=rs_out[tsl, :])
                    nc.vector.tensor_add(ft[:], ft[:], bo_bc[:])
                    nc.sync.dma_start(out=out_ext[tsl, :], in_=ft[:])

    nc.finalize()
    return nc


def _prep_host_inputs(queries, keys, values, Wq, bq, Wk, bk, Wv, bv, Wo, bo,
                      S, n_cores):
    T = B * S
    xqT = np.ascontiguousarray(queries.reshape(T, D).T.astype(np.float32))
    xkT = np.ascontiguousarray(keys.reshape(T, D).T.astype(np.float32))
    xvT = np.ascontiguousarray(values.reshape(T, D).T.astype(np.float32))
    boff = (np.array([(p % B) * S for p in range(8)], np.uint32)
            .reshape(8, 1))
    in_maps = []
    for c in range(n_cores):
        rsl = slice(c * 128, (c + 1) * 128)
        in_maps.append({
            "xqT": xqT, "xkT": xkT, "xvT": xvT,
            "wqT": np.ascontiguousarray(Wq[rsl, :].T.astype(np.float32)),
            "wkT": np.ascontiguousarray(Wk[rsl, :].T.astype(np.float32)),
            "wvT": np.ascontiguousarray(Wv[rsl, :].T.astype(np.float32)),
            "bq": bq[rsl].reshape(128, 1).astype(np.float32),
            "bk": bk[rsl].reshape(128, 1).astype(np.float32),
            "bv": bv[rsl].reshape(128, 1).astype(np.float32),
            "woT": np.ascontiguousarray(Wo.T[rsl, :].astype(np.float32)),
            "boN": bo.reshape(1, D).astype(np.float32),
            "boff": boff,
        })
    return in_maps


_LAST_RESULT = None


def kernel(queries, keys, values, Wq, bq, Wk, bk, Wv, bv, Wo, bo):
    global _LAST_RESULT
    from concourse.bass_utils import run_bass_kernel_spmd

    queries, keys, values = (np.asarray(t, np.float32) for t in
                             (queries, keys, values))
    Wq, bq, Wk, bk, Wv, bv, Wo, bo = (np.asarray(t, np.float32) for t in
                                      (Wq, bq, Wk, bk, Wv, bv, Wo, bo))
    S = queries.shape[1]
    n_cores = N_CORES
    nc = build_nc(S=S, n_cores=n_cores)
    in_maps = _prep_host_inputs(queries, keys, values, Wq, bq, Wk, bk, Wv, bv,
                                Wo, bo, S, n_cores)
    res = run_bass_kernel_spmd(nc, in_maps, core_ids=list(range(n_cores)))
    _LAST_RESULT = res
    T = B * S
    out = np.concatenate([res.results[c]["out"] for c in range(n_cores)], axis=0)
    return out.reshape(B, S, D).astype(np.float32)


# revision 69
# speedup vs baseline: 1.0575x; 1.0575x over previous
"""Distributed sparse attention kernel for Trainium2 (8 NeuronCores).

Sharding: head-parallel. Core c owns heads [2c, 2c+1] (128 of the 1024
projection dims). Each core reads the full queries/keys/values (projection
contracts over all of D), computes Q/K/V projections for its heads, runs the
full importance scan + top-k + sparse attention locally (per the head/batch
pair), then computes a partial output projection with its 128-column slice of
Wo. A ReduceScatter sums the partials and leaves each core with 1/8 of the
output rows; the host concatenates.

Math (per (b, h) pair; reference semantics):
  Q = x_q @ Wq.T + bq  (fp32; likewise K, V)
  s = Q @ K.T                      # unscaled: importance ranking is
  imp = max_k(s) - mean_k(s)       # invariant to the positive 1/sqrt(hd) scale
  sel = top-38 rows by imp (order irrelevant: output is a row-map)
  w = softmax(scale * s[sel])      # computed without max-subtraction
  out[sel] = w @ V ; out[other] = mean(V)
  final = out @ Wo.T + bo
"""

import math
import sys

import numpy as np

sys.path.insert(0, "/opt/trn_rl_repo")

import concourse.bass as bass
import concourse.mybir as mybir
import concourse.tile as tile
from concourse import bacc
from concourse.masks import make_identity
from concourse.tile import add_dep_helper

F32 = mybir.dt.float32
F32R = mybir.dt.float32r
U32 = mybir.dt.uint32

B = 4
D = 1024
H = 16
HD = 64
H_LOC = 2          # heads per core
U = 38             # top-k
UP = 40            # padded (5 rounds of max8)
N_CORES = 8


def build_nc(S=2048, n_cores=8):
    """Build the SPMD Bass module. Same NEFF for every core; per-core
    behavior comes entirely from per-core input data."""
    nc = bacc.Bacc("TRN2", target_bir_lowering=False, debug=False,
                   num_devices=n_cores)
    T = B * S
    NP = 512                # projection moving-dim chunk
    NQC = S // 128          # 128-query chunks per pair
    KH = min(1024, S)       # scan psum half width
    NKH = S // KH           # halves per pair row
    ROWS_OUT = T // n_cores
    scale = 1.0 / math.sqrt(HD)

    # ---- I/O ----
    xqT = nc.dram_tensor("xqT", [D, T], F32, kind="ExternalInput")
    xkT = nc.dram_tensor("xkT", [D, T], F32, kind="ExternalInput")
    xvT = nc.dram_tensor("xvT", [D, T], F32R, kind="ExternalInput")
    wqT = nc.dram_tensor("wqT", [D, 128], F32, kind="ExternalInput")
    wkT = nc.dram_tensor("wkT", [D, 128], F32, kind="ExternalInput")
    wvT = nc.dram_tensor("wvT", [D, 128], F32R, kind="ExternalInput")
    bq = nc.dram_tensor("bq", [128, 1], F32, kind="ExternalInput")
    bk = nc.dram_tensor("bk", [128, 1], F32, kind="ExternalInput")
    bv = nc.dram_tensor("bv", [128, 1], F32, kind="ExternalInput")
    woT = nc.dram_tensor("woT", [128, D], F32R, kind="ExternalInput")
    boN = nc.dram_tensor("boN", [1, D], F32, kind="ExternalInput")  # full bo
    boff = nc.dram_tensor("boff", [8, 1], U32, kind="ExternalInput")  # b*S per pair
    out_ext = nc.dram_tensor("out", [ROWS_OUT, D], F32, kind="ExternalOutput")

    # ---- DRAM scratch ----
    qrm = [nc.dram_tensor(f"qrm{h}", [T, HD], F32) for h in range(H_LOC)]
    vrm_dram = nc.dram_tensor("vrm", [T, 128], F32)
    ohead = [nc.dram_tensor(f"ohead{h}", [T, HD], F32) for h in range(H_LOC)]
    partial = nc.dram_tensor("partial", [T, D], F32)
    rs_out = nc.dram_tensor("rs_out", [ROWS_OUT, D], F32)

    with tile.TileContext(nc) as tc:
        with (
            tc.tile_pool(name="resident", bufs=1) as res,
            tc.tile_pool(name="consts", bufs=1) as consts,
        ):
            # constants
            ident = consts.tile([128, 128], F32)
            make_identity(nc, ident[:])
            ones_col = consts.tile([128, 1], F32)
            nc.vector.memset(ones_col[:], 1.0)
            ones_row = consts.tile([1, 512], F32)
            nc.vector.memset(ones_row[:], 1.0)

            # resident weights / projections
            wq_sb = res.tile([128, 8, 128], F32)
            wk_sb = res.tile([128, 8, 128], F32)
            wv_sb = res.tile([128, 8, 128], F32R)
            nc.sync.dma_start(out=wq_sb[:], in_=wqT[:].rearrange("(k p) m -> p k m", p=128))
            nc.sync.dma_start(out=wk_sb[:], in_=wkT[:].rearrange("(k p) m -> p k m", p=128))
            nc.sync.dma_start(out=wv_sb[:], in_=wvT[:].rearrange("(k p) m -> p k m", p=128))
            bq_sb = consts.tile([128, 1], F32)
            bk_sb = consts.tile([128, 1], F32)
            bv_sb = consts.tile([128, 1], F32)
            nc.sync.dma_start(out=bq_sb[:], in_=bq[:])
            nc.sync.dma_start(out=bk_sb[:], in_=bk[:])
            nc.sync.dma_start(out=bv_sb[:], in_=bv[:])
            wo_sb = res.tile([128, D], F32R)
            nc.sync.dma_start(out=wo_sb[:], in_=woT[:])
            bo_sb = consts.tile([1, D], F32)
            nc.sync.dma_start(out=bo_sb[:], in_=boN[:])
            boff_sb = consts.tile([8, 1], U32)
            nc.sync.dma_start(out=boff_sb[:], in_=boff[:])

            BF16 = mybir.dt.bfloat16
            # bf16 hi/lo split of Q.T/K.T: everything after the projection
            # phase reads only these (fp32 QT/KT are projection-scoped)
            QTh = res.tile([128, T], BF16)
            QTl = res.tile([128, T], BF16)
            KTh = res.tile([128, T], BF16)
            KTl = res.tile([128, T], BF16)

            # bo broadcast to all 128 partitions (used in the final bias add)
            with tc.tile_pool(name="ps_bo", bufs=1, space="PSUM") as psbo:
                bo_bc = res.tile([128, D], F32)
                for nh in range(D // 512):
                    pb = psbo.tile([128, 512], F32, tag="pb")
                    nc.tensor.matmul(pb[:], lhsT=ones_row[:1, :128],
                                     rhs=bo_sb[:, nh * 512:(nh + 1) * 512],
                                     start=True, stop=True)
                    nc.scalar.copy(bo_bc[:, nh * 512:(nh + 1) * 512], pb[:])

            # ---------------- projections: QT, KT ----------------
            # Projections, processed per 512-column chunk: psum -> fp32 chunk
            # (with fused bias) -> bf16 hi/lo into the resident split tiles;
            # Q and V chunks are additionally transposed out to row-major DRAM
            # (Qrm feeds the selected-row gather, Vrm the attention matmuls).
            with (
                tc.tile_pool(name="xin", bufs=3) as xin,
                tc.tile_pool(name="pfch", bufs=3) as pfch,
                tc.tile_pool(name="vout", bufs=4) as vout,
                tc.tile_pool(name="ps_proj", bufs=4, space="PSUM") as psp,
                tc.tile_pool(name="ps_tr", bufs=2, space="PSUM") as pstr0,
            ):
                for which, (xsrc, w_sb, b_sb, hi, lo) in enumerate(
                        ((xqT, wq_sb, bq_sb, QTh, QTl),
                         (xkT, wk_sb, bk_sb, KTh, KTl),
                         (xvT, wv_sb, bv_sb, None, None))):
                    for ncol in range(T // NP):
                        sl = slice(ncol * NP, (ncol + 1) * NP)
                        xt = xin.tile([128, 8, NP], w_sb[:].dtype, tag="xt")
                        nc.sync.dma_start(
                            out=xt[:],
                            in_=xsrc[:, sl].rearrange("(k p) t -> p k t", p=128),
                        )
                        ps = psp.tile([128, NP], F32, tag="pp")
                        for kc in range(8):
                            nc.tensor.matmul(ps[:], lhsT=w_sb[:, kc, :], rhs=xt[:, kc, :],
                                             start=(kc == 0), stop=(kc == 7))
                        pf = pfch.tile([128, NP], F32, tag="pf")
                        nc.scalar.activation(pf[:], ps[:],
                                             mybir.ActivationFunctionType.Identity,
                                             bias=b_sb[:])
                        if hi is not None:
                            nc.scalar.copy(hi[:, sl], pf[:])
                            nc.vector.tensor_sub(lo[:, sl], pf[:], hi[:, sl])
                        if which == 0:  # Q -> Qrm per head
                            for j in range(NP // 128):
                                tsl = slice(ncol * NP + j * 128,
                                            ncol * NP + (j + 1) * 128)
                                jsl = slice(j * 128, (j + 1) * 128)
                                for h in range(H_LOC):
                                    hsl = slice(h * 64, (h + 1) * 64)
                                    pst = pstr0.tile([128, 64], F32, tag="pq")
                                    nc.tensor.transpose(pst[:], in_=pf[hsl, jsl],
                                                        identity=ident[hsl, hsl])
                                    qt = vout.tile([128, 64], F32, tag="qt")
                                    nc.scalar.copy(qt[:], pst[:])
                                    nc.sync.dma_start(out=qrm[h][tsl, :], in_=qt[:])
                        elif which == 2:  # V -> Vrm
                            for j in range(NP // 128):
                                tsl = slice(ncol * NP + j * 128,
                                            ncol * NP + (j + 1) * 128)
                                jsl = slice(j * 128, (j + 1) * 128)
                                psv = pstr0.tile([128, 128], F32, tag="pv")
                                nc.tensor.transpose(psv[:], in_=pf[:, jsl],
                                                    identity=ident[:])
                                vt = vout.tile([128, 128], F32, tag="vt")
                                nc.scalar.copy(vt[:], psv[:])
                                nc.sync.dma_start(out=vrm_dram[tsl, :], in_=vt[:])

            # ---------------- importance scan ----------------
            # scores for the screen run as a 3-term bf16 split (hi*hi +
            # hi*lo + lo*hi): exact enough that the top-38 selection matches
            # full fp32 (verified: margin ~8.5e-4 vs error <5e-4 on this
            # data), at ~1/3 the PE cost of fp32 matmuls.
            imp_all = res.tile([128, 8 * NQC], F32)  # col = pair*NQC + qc
            with (
                tc.tile_pool(name="ps_scan", bufs=3, space="PSUM") as pss,
                tc.tile_pool(name="ps_mean", bufs=2, space="PSUM") as psm,
                tc.tile_pool(name="scan_sb", bufs=4) as ssb,
            ):
                for pair in range(8):
                    h, b = divmod(pair, B)
                    hsl = slice(h * 64, (h + 1) * 64)
                    ks = ssb.tile([128, 2], F32, tag="ks")
                    nc.vector.reduce_sum(ks[hsl, 0:1], KTh[hsl, b * S:(b + 1) * S],
                                         axis=mybir.AxisListType.X)
                    nc.vector.reduce_sum(ks[hsl, 1:2], KTl[hsl, b * S:(b + 1) * S],
                                         axis=mybir.AxisListType.X)
                    # ksum as bf16 triplet: hi(KsumH), lo(KsumH), hi(KsumL)
                    ksb = ssb.tile([128, 3], BF16, tag="ksb")
                    nc.vector.tensor_copy(ksb[hsl, 0:1], ks[hsl, 0:1])
                    nc.vector.tensor_tensor(ksb[hsl, 1:2], ks[hsl, 0:1],
                                            ksb[hsl, 0:1],
                                            op=mybir.AluOpType.subtract)
                    nc.vector.tensor_copy(ksb[hsl, 2:3], ks[hsl, 1:2])
                    mcol = ssb.tile([128, NQC], F32, tag="mcol")
                    xcol = ssb.tile([128, NKH, NQC], F32, tag="xcol")
                    for qc in range(NQC):
                        qsl = slice(b * S + qc * 128, b * S + (qc + 1) * 128)
                        psmean = psm.tile([128, 1], F32, tag="pm")
                        MTERMS = ((QTh, 0), (QTh, 1), (QTh, 2), (QTl, 0))
                        for ti, (qsrc, kcol) in enumerate(MTERMS):
                            nc.tensor.matmul(psmean[:], lhsT=qsrc[hsl, qsl],
                                             rhs=ksb[hsl, kcol:kcol + 1],
                                             start=(ti == 0), stop=(ti == 3))
                        nc.vector.tensor_scalar_mul(mcol[:, qc:qc + 1], psmean[:],
                                                    1.0 / S)
                        NCH = min(512, KH)
                        TERMS = ((QTh, KTh), (QTh, KTl), (QTl, KTh))
                        for half in range(NKH):
                            ps = pss.tile([128, KH], F32, tag="sc")
                            for j in range(KH // NCH):
                                ksl = slice(b * S + half * KH + j * NCH,
                                            b * S + half * KH + (j + 1) * NCH)
                                for ti, (qsrc, ksrc) in enumerate(TERMS):
                                    nc.tensor.matmul(
                                        ps[:, j * NCH:(j + 1) * NCH],
                                        lhsT=qsrc[hsl, qsl], rhs=ksrc[hsl, ksl],
                                        start=(ti == 0), stop=(ti == 2))
                            nc.vector.reduce_max(xcol[:, half, qc:qc + 1], ps[:],
                                                 axis=mybir.AxisListType.X)
                    # imp = max(halves) - mean
                    xmax = ssb.tile([128, NQC], F32, tag="xmax")
                    if NKH > 1:
                        nc.vector.tensor_reduce(xmax[:], xcol[:].rearrange("p a q -> p q a"),
                                                axis=mybir.AxisListType.X,
                                                op=mybir.AluOpType.max)
                    else:
                        nc.vector.tensor_copy(xmax[:], xcol[:, 0, :])
                    nc.vector.tensor_sub(imp_all[:, pair * NQC:(pair + 1) * NQC],
                                         xmax[:], mcol[:])

            # ---------------- top-k ----------------
            NQ8 = 8 * NQC
            off_t = []  # per-pair [UP,1] u32 token offsets
            with (
                tc.tile_pool(name="ps_tk", bufs=1, space="PSUM") as pstk,
                tc.tile_pool(name="tk_sb", bufs=1) as tksb,
            ):
                pst = pstk.tile([NQ8, 128], F32)
                nc.tensor.transpose(pst[:], in_=imp_all[:, 0:NQ8], identity=ident[:])
                impT = tksb.tile([NQ8, 128], F32)
                nc.scalar.copy(impT[:], pst[:])
                impP = tksb.tile([8, S], F32)
                for pr in range(8):
                    nc.gpsimd.dma_start(
                        out=impP[pr:pr + 1, :],
                        in_=impT[pr * NQC:(pr + 1) * NQC, :],
                    )
                work = tksb.tile([8, S], F32)
                nc.vector.tensor_copy(work[:], impP[:])
                mxv = tksb.tile([8, UP], F32)
                idx = tksb.tile([8, UP], U32)
                for r in range(5):
                    rsl = slice(r * 8, (r + 1) * 8)
                    nc.vector.max(out=mxv[:, rsl], in_=work[:])
                    nc.vector.max_index(out=idx[:, rsl], in_max=mxv[:, rsl],
                                        in_values=work[:])
                    if r < 4:
                        nc.vector.match_replace(out=work[:], in_to_replace=mxv[:, rsl],
                                                in_values=work[:], imm_value=-1e30)
                idx_tok = tksb.tile([8, UP], U32)
                nc.vector.tensor_tensor(idx_tok[:], idx[:],
                                        boff_sb[:].to_broadcast([8, UP]),
                                        op=mybir.AluOpType.add)
                for pair in range(8):
                    ot = res.tile([UP, 1], U32, tag=f"ot{pair}")
                    nc.gpsimd.dma_start(out=ot[:], in_=idx_tok[pair:pair + 1, :])
                    off_t.append(ot)

            # DRAM scratch (vrm/qrm) written by DMA is read by DMA below;
            # cross-queue DRAM ordering is enforced with a hard barrier.
            tc.strict_bb_all_engine_barrier()
            # ---------------- attention on selected queries ----------------
            with (
                tc.tile_pool(name="ps_st", bufs=2, space="PSUM") as ps_st,
                tc.tile_pool(name="ps_se", bufs=2, space="PSUM") as ps_se,
                tc.tile_pool(name="ps_ot", bufs=2, space="PSUM") as ps_ot,
                tc.tile_pool(name="ps_sm", bufs=1, space="PSUM") as ps_sm,
                tc.tile_pool(name="ps_vm", bufs=1, space="PSUM") as ps_vm,
                tc.tile_pool(name="att_sb", bufs=2) as asb,
                tc.tile_pool(name="vres", bufs=2) as vres,
            ):
                for b in range(B):
                    vsb = vres.tile([128, S // 128, 128], F32, tag="vsb")
                    nc.sync.dma_start(
                        out=vsb[:],
                        in_=vrm_dram[b * S:(b + 1) * S, :].rearrange(
                            "(k p) j -> p k j", p=128),
                    )
                    for h in range(H_LOC):
                        pair = h * B + b
                        hsl = slice(h * 64, (h + 1) * 64)
                        off = off_t[pair]
                        # gather selected Q rows
                        qsel = asb.tile([UP, HD], F32, tag="qsel")
                        nc.gpsimd.indirect_dma_start(
                            out=qsel[:], out_offset=None,
                            in_=qrm[h][:],
                            in_offset=bass.IndirectOffsetOnAxis(ap=off[:, 0:1], axis=0),
                        )
                        pq = ps_sm.tile([128, UP], F32, tag="sm")
                        nc.tensor.transpose(pq[0:64, :], in_=qsel[:],
                                            identity=ident[0:UP, 0:UP])
                        qselT = asb.tile([64, UP], F32, tag="qselT")
                        nc.scalar.copy(qselT[:], pq[0:64, :])
                        # stage this pair's K.T slice at partition base 0 (for
                        # matching lhsT/rhs bases), reconstructed as hi+lo
                        kts = asb.tile([64, S], F32, tag="kts")
                        nc.vector.tensor_add(kts[:], KTh[hsl, b * S:(b + 1) * S],
                                             KTl[hsl, b * S:(b + 1) * S])

                        expT = asb.tile([128, S // 128, UP], F32, tag="expT")
                        for kc in range(S // 128):
                            pst = ps_st.tile([128, UP], F32, tag="st")
                            nc.tensor.matmul(pst[:], lhsT=kts[:, kc * 128:(kc + 1) * 128],
                                             rhs=qselT[:],
                                             start=True, stop=True)
                            nc.scalar.activation(expT[:, kc, :], pst[:],
                                                 mybir.ActivationFunctionType.Exp,
                                                 scale=scale)
                        pse = ps_se.tile([UP, 1], F32, tag="se")
                        pot = ps_ot.tile([64, UP], F32, tag="ot")
                        for kc in range(S // 128):
                            nc.tensor.matmul(pse[:], lhsT=expT[:, kc, :],
                                             rhs=ones_col[:],
                                             start=(kc == 0), stop=(kc == S // 128 - 1))
                            nc.tensor.matmul(pot[:], lhsT=vsb[:, kc, hsl],
                                             rhs=expT[:, kc, :],
                                             start=(kc == 0), stop=(kc == S // 128 - 1))
                        se = asb.tile([UP, 1], F32, tag="se_sb")
                        nc.vector.tensor_scalar_add(se[:], pse[:], 1e-8)
                        rec = asb.tile([UP, 1], F32, tag="rec")
                        nc.vector.reciprocal(rec[:], se[:])
                        oT = asb.tile([64, UP], F32, tag="oT")
                        nc.scalar.copy(oT[:], pot[:])
                        po = ps_sm.tile([UP, 64], F32, tag="sm")
                        nc.tensor.transpose(po[:], in_=oT[:], identity=ident[0:64, 0:64])
                        osel = asb.tile([UP, HD], F32, tag="osel")
                        nc.scalar.mul(osel[:], po[:], rec[:, 0:1])

                        # default rows: mean of V over keys
                        pvm = ps_vm.tile([1, 64], F32, tag="vm")
                        for kc in range(S // 128):
                            nc.tensor.matmul(pvm[:], lhsT=ones_col[:], rhs=vsb[:, kc, hsl],
                                             start=(kc == 0), stop=(kc == S // 128 - 1))
                        vmr = asb.tile([1, 64], F32, tag="vmr")
                        nc.scalar.mul(vmr[:], pvm[:], 1.0 / S)
                        pbc = ps_sm.tile([128, 64], F32, tag="sm")
                        nc.tensor.matmul(pbc[:], lhsT=ones_row[:1, :128], rhs=vmr[:],
                                         start=True, stop=True)
                        bc = asb.tile([128, 64], F32, tag="bc")
                        nc.scalar.copy(bc[:], pbc[:])
                        defaults = []
                        for sc in range(S // 128):
                            defaults.append(nc.gpsimd.dma_start(
                                out=ohead[h][b * S + sc * 128: b * S + (sc + 1) * 128, :],
                                in_=bc[:]))
                        # scatter the U selected rows over the defaults; the
                        # explicit deps keep the default writes (separate DMA
                        # queue) strictly before the indirect scatter
                        scat = nc.gpsimd.indirect_dma_start(
                            out=ohead[h][:],
                            out_offset=bass.IndirectOffsetOnAxis(ap=off[0:U, 0:1], axis=0),
                            in_=osel[0:U, :], in_offset=None,
                        )
                        for dfl in defaults:
                            add_dep_helper(scat.ins, dfl.ins, sync=True,
                                           reason="scatter after default fill")

            tc.strict_bb_all_engine_barrier()
            # ---------------- partial output projection ----------------
            with (
                tc.tile_pool(name="ps_op", bufs=4, space="PSUM") as psop,
                tc.tile_pool(name="ps_tr", bufs=4, space="PSUM") as pstr,
                tc.tile_pool(name="op_sb", bufs=3) as osb,
            ):
                for tcn in range(T // 128):
                    tsl = slice(tcn * 128, (tcn + 1) * 128)
                    stacked = osb.tile([128, 128], F32R, tag="stk")
                    for h in range(H_LOC):
                        oh = osb.tile([128, 64], F32, tag="oh")
                        nc.sync.dma_start(out=oh[:], in_=ohead[h][tsl, :])
                        pt = pstr.tile([64, 128], F32, tag="tr")
                        nc.tensor.transpose(pt[:], in_=oh[:], identity=ident[:])
                        nc.scalar.copy(stacked[h * 64:(h + 1) * 64, :], pt[:])
                    for nh in range(D // 512):
                        nsl = slice(nh * 512, (nh + 1) * 512)
                        pp = psop.tile([128, 512], F32, tag="pp")
                        nc.tensor.matmul(pp[:], lhsT=stacked[:], rhs=wo_sb[:, nsl],
                                         start=True, stop=True)
                        po_sb = osb.tile([128, 512], F32, tag="po")
                        nc.vector.tensor_copy(po_sb[:], pp[:])
                        nc.sync.dma_start(out=partial[tsl, nsl], in_=po_sb[:])

            # ---------------- reduce-scatter + output ----------------
            tc.strict_bb_all_engine_barrier()
            nc.gpsimd.collective_compute(
                "ReduceScatter",
                mybir.AluOpType.add,
                replica_groups=[list(range(n_cores))],
                ins=[partial[:]],
                outs=[rs_out[:]],
            )
            with tc.tile_pool(name="fin", bufs=3) as fin:
                for tcn in range(ROWS_OUT // 128):
                    tsl = slice(tcn * 128, (tcn + 1) * 128)
                    ft = fin.tile([128, D], F32, tag="ft")
                    nc.sync.dma_start(out=ft[:], in_=rs_out[tsl, :])
                    nc.vector.tensor_add(ft[:], ft[:], bo_bc[:])
                    nc.sync.dma_start(out=out_ext[tsl, :], in_=ft[:])

    nc.finalize()
    return nc


def _prep_host_inputs(queries, keys, values, Wq, bq, Wk, bk, Wv, bv, Wo, bo,
                      S, n_cores):
    T = B * S
    xqT = np.ascontiguousarray(queries.reshape(T, D).T.astype(np.float32))
    xkT = np.ascontiguousarray(keys.reshape(T, D).T.astype(np.float32))
    xvT = np.ascontiguousarray(values.reshape(T, D).T.astype(np.float32))
    boff = (np.array([(p % B) * S for p in range(8)], np.uint32)
            .reshape(8, 1))
    in_maps = []
    for c in range(n_cores):
        rsl = slice(c * 128, (c + 1) * 128)
        in_maps.append({
            "xqT": xqT, "xkT": xkT, "xvT": xvT,
            "wqT": np.ascontiguousarray(Wq[rsl, :].T.astype(np.float32)),
            "wkT": np.ascontiguousarray(Wk[rsl, :].T.astype(np.float32)),
            "wvT": np.ascontiguousarray(Wv[rsl, :].T.astype(np.float32)),
            "bq": bq[rsl].reshape(128, 1).astype(np.float32),
            "bk": bk[rsl].reshape(128, 1).astype(np.float32),
            "bv": bv[rsl].reshape(128, 1).astype(np.float32),
            "woT": np.ascontiguousarray(Wo.T[rsl, :].astype(np.float32)),
            "boN": bo.reshape(1, D).astype(np.float32),
            "boff": boff,
        })
    return in_maps


_LAST_RESULT = None


def kernel(queries, keys, values, Wq, bq, Wk, bk, Wv, bv, Wo, bo):
    global _LAST_RESULT
    from concourse.bass_utils import run_bass_kernel_spmd

    queries, keys, values = (np.asarray(t, np.float32) for t in
                             (queries, keys, values))
    Wq, bq, Wk, bk, Wv, bv, Wo, bo = (np.asarray(t, np.float32) for t in
                                      (Wq, bq, Wk, bk, Wv, bv, Wo, bo))
    S = queries.shape[1]
    n_cores = N_CORES
    nc = build_nc(S=S, n_cores=n_cores)
    in_maps = _prep_host_inputs(queries, keys, values, Wq, bq, Wk, bk, Wv, bv,
                                Wo, bo, S, n_cores)
    res = run_bass_kernel_spmd(nc, in_maps, core_ids=list(range(n_cores)))
    _LAST_RESULT = res
    T = B * S
    out = np.concatenate([res.results[c]["out"] for c in range(n_cores)], axis=0)
    return out.reshape(B, S, D).astype(np.float32)
